# revision 35
# baseline (speedup 1.0000x reference)
"""Trainium2 8-core attention kernel (nn_Attention_19954418057485).

Sharding: heads are split across the 8 cores (2 heads = 128 channels
each); every core processes both batch elements for its heads.  Four
AllToAlls over all 8 cores (one per 1024-row chunk, each overlapped
with compute) swap the channel axis for the row axis, so each core
finishes the full output projection for 512 rows (4 x 128, one 128-row
block per chunk) of the flattened (B*N, C) output.

Per-core pipeline (matmuls on PE in bf16, exp on ACT, elementwise DVE):
  x^T (bf16)  --PE-->  q,k (rows,ch) + v^T        [QKV projection]
  q,k: LayerNorm (d=64) + RoPE (bf16 DVE ops), then PE transposes to
  q^T,k^T [ch, n]; v^T -> V [n, ch] with a ones column appended.
  per (batch, head): S^T = K Q^T, exp(S/8) on ACT (no max-subtraction
  needed: layernormed q,k bound |scores| <= 8), AV accumulates
  V_ext^T @ expS^T giving out^T rows 0..63 plus the softmax denominator
  in row 64 (from the ones column).  Normalization per (batch, qt):
  both heads' denominator rows -> one PE block-outer-product broadcast
  [128, 512] -> 1/x via ACT exp(-ln(x)) at full 128-lane occupancy ->
  two DVE multiplies.

Instruction emission interleaves batch-0 attention (ACT-bound) with the
batch-1 preamble (DVE-bound); output-projection matmuls are only
emitted at points where their AllToAll is provably complete so the PE's
in-order queue never head-of-line blocks on a collective.
"""
import sys

if "/opt/trn_rl_repo" not in sys.path:
    sys.path.insert(0, "/opt/trn_rl_repo")

import numpy as np
import ml_dtypes

import concourse.bass as bass
import concourse.tile as tile
from concourse import mybir
from concourse.bass_utils import run_bass_kernel_spmd

N_CORES = 8
B, N, C, H = 2, 2048, 1024, 16
D = 64
HPC = H // N_CORES          # heads per core = 2
CPC = HPC * D               # channels per core = 128
NTOT = B * N                # 4096 flattened rows
RPC = NTOT // N_CORES       # output rows per core = 512
QROW = 128                  # rows per core per collective chunk
EPS = 1e-6

BF16 = mybir.dt.bfloat16
F32 = mybir.dt.float32
AF = mybir.ActivationFunctionType
OP = mybir.AluOpType
AX = mybir.AxisListType


def _split_excess_waits(nc, max_waits=1):
    """walrus rejects instructions with more than a couple of sem-wait
    commands; split extras onto preceding same-engine NoOps."""
    for fn in nc.m.functions:
        for blk in fn.blocks:
            new_insts = []
            for ins in blk.instructions:
                si = ins.sync_info
                ow = list(si.on_wait) if si is not None and si.on_wait else []
                if len(ow) > max_waits:
                    head = ow[: len(ow) - max_waits]
                    rest = ow[len(ow) - max_waits:]
                    for i in range(0, len(head), max_waits):
                        new_insts.append(mybir.InstNoOp(
                            name=f"{ins.name}_ws{i}",
                            engine=ins.engine,
                            ins=[], outs=[],
                            sync_info=mybir.SyncInfo(
                                on_wait=head[i:i + max_waits], on_update=[]),
                        ))
                    ins.sync_info = mybir.SyncInfo(
                        on_wait=rest, on_update=list(si.on_update or []))
                new_insts.append(ins)
            blk.instructions = new_insts


def build():
    nc = bass.Bass("TRN2", target_bir_lowering=False, debug=False,
                   num_devices=N_CORES)
    xT_d = nc.dram_tensor("xT", (C, NTOT), BF16, kind="ExternalInput")
    wqkv_d = nc.dram_tensor("wqkvT", (C, 3 * CPC), BF16, kind="ExternalInput")
    wpT_d = nc.dram_tensor("wpT", (C, C), BF16, kind="ExternalInput")
    bias_d = nc.dram_tensor("biasb", (128, C), F32, kind="ExternalInput")
    cos_d = nc.dram_tensor("cosd", (N, D), BF16, kind="ExternalInput")
    sin_d = nc.dram_tensor("sind", (N, D), BF16, kind="ExternalInput")
    ident_d = nc.dram_tensor("identd", (128, 128), F32, kind="ExternalInput")
    bones_d = nc.dram_tensor("bonesd", (128, 256), BF16, kind="ExternalInput")
    out_d = nc.dram_tensor("out", (RPC, C), F32, kind="ExternalOutput")
    import os as _os
    _DBG = bool(_os.environ.get("KBG_DEBUG"))
    if _DBG:
        dbg_ao_d = nc.dram_tensor("dbg_ao", (128, NTOT), BF16,
                                  kind="ExternalOutput")
        dbg_qp_d = nc.dram_tensor("dbg_qp", (128, 2, NTOT), BF16,
                                  kind="ExternalOutput")
        dbg_kt_d = nc.dram_tensor("dbg_kt", (128, NTOT), BF16,
                                  kind="ExternalOutput")
        dbg_vx_d = nc.dram_tensor("dbg_vx", (128, 32, HPC, 128), BF16,
                                  kind="ExternalOutput")

    with tile.TileContext(nc) as tc:
        with tc.tile_pool(name="consts", bufs=1) as consts, \
             tc.tile_pool(name="xload", bufs=3) as xload, \
             tc.tile_pool(name="qkrp", bufs=2) as qkrp, \
             tc.tile_pool(name="freqs", bufs=2) as freqs, \
             tc.tile_pool(name="work", bufs=3) as work, \
             tc.tile_pool(name="small", bufs=2) as small, \
             tc.tile_pool(name="exps", bufs=6) as expp, \
             tc.tile_pool(name="norm", bufs=2) as normp, \
             tc.tile_pool(name="ps", bufs=2, space="PSUM") as ps, \
             tc.tile_pool(name="psS", bufs=2, space="PSUM") as psSp, \
             tc.tile_pool(name="psav", bufs=2, space="PSUM") as psav, \
             tc.tile_pool(name="dram", bufs=1, space="DRAM") as dram:

            # ---- constants (ordered so the first QKV matmul can start
            # as early as possible: wqkv first, bias/wp deferred) ------
            wqkv_sb = consts.tile([128, 8, 3 * CPC], BF16)
            nc.sync.dma_start(wqkv_sb[:],
                              wqkv_d.ap().rearrange("(co p) k -> p co k", p=128))
            ident_f = consts.tile([128, 128], F32)
            # row 0: [0:128]=head0 column-block ones, [128:256]=head1's;
            # rows 1-127 zero so the norm broadcast matmul runs K=128
            # (same 128x128 PE tiling mode as AV/QKV -- no mode-switch
            # drain mid-attention).
            bones_sb = consts.tile([128, 256], BF16)
            # zero-padded denominator staging: row 0 of each half carries
            # the per-head softmax denominators, rows 1-127 stay zero.
            den_z = consts.tile([128, 2, 512], BF16)
            wp_sb = consts.tile([128, 8, C], BF16)      # DMA deferred
            bias_sb = consts.tile([128, C], F32)        # DMA deferred
            # all RoPE tables resident: batch 1 (tp2/3) reuses the same
            # positions as batch 0 (tp0/1), so one load serves all four tps
            cos_sb = consts.tile([128, 16, D], BF16)    # DMA deferred
            sin_sb = consts.tile([128, 16, D], BF16)    # DMA deferred
            cos_r = cos_d.ap().rearrange("(c p) d -> p c d", p=128)
            sin_r = sin_d.ap().rearrange("(c p) d -> p c d", p=128)
            identr = consts.tile([128, 128], BF16)

            # ---- persistent tensors ---------------------------------------
            # k transposed: [ch (both heads), b*N+n]
            kT = consts.tile([128, NTOT], BF16)
            # q transposed, zero-padded per head: qpad[:, h] holds head h's
            # q rows in its 64-channel band and ZEROS in the other band, so
            # the S matmuls contract K=128 (same 128x128 PE tiling mode as
            # every other matmul -- no mode-switch drains; the padded rows
            # multiply k's other-head channels by zero).
            qpad = consts.tile([128, 2, NTOT], BF16)
            # V with ones column, padded to 128 so the AV ldweights is a
            # full-128-column load (FWL-eligible): [n%128, chunk, head,
            # 64 d + 1 one + 63 zeros]
            vext = consts.tile([128, 32, HPC, 128], BF16)
            attn_outT = consts.tile([128, NTOT], BF16)
            # zero the padded tensors FIRST (before any producer writes are
            # emitted) with simple contiguous memsets: every later write is
            # WAW-ordered after these, and strided partial memsets were
            # observed to corrupt neighbouring columns.
            nc.vector.memset(den_z[:], 0.0)
            nc.vector.memset(qpad[:], 0.0)
            nc.vector.memset(vext[:], 0.0)

            def emit_late_consts():
                """everything not needed by the first QKV matmuls: emitted
                after pre_gen(0) so its DMA triggers don't delay xt0."""
                nc.sync.dma_start(ident_f[:], ident_d.ap())
                nc.sync.dma_start(bones_sb[:], bones_d.ap())
                nc.scalar.activation(identr[:], ident_f[:], AF.Copy)
                nc.scalar.activation(
                    vext[:, :, :, 64:65],
                    ident_f[:, 0:64].rearrange("p (a b c) -> p a b c",
                                               a=32, b=2),
                    AF.Identity, scale=0.0, bias=1.0)

            xT_r = xT_d.ap().rearrange("(co p) n -> p co n", p=128)
            state = {}

            def load_xt(tp):
                """trigger tp's x-chunk DMA (split so ns 0-3 can start
                after the first MB)."""
                xt = xload.tile([128, 8, 1024], BF16, tag="xt",
                                name=f"xt{tp}")
                nc.sync.dma_start(xt[:, :, 0:512],
                                  xT_r[:, :, 1024 * tp:1024 * tp + 512])
                nc.sync.dma_start(xt[:, :, 512:1024],
                                  xT_r[:, :, 1024 * tp + 512:1024 * (tp + 1)])
                return xt

            def pre_gen(tp, xt):
                """QKV proj + LN + RoPE for rows [tp*1024, (tp+1)*1024)."""
                qk_nd = work.tile([128, 8, 4, D], BF16, tag="qknd",
                                  name=f"qknd{tp}")
                for ns in range(8):
                    pj = ps.tile([128, 3 * CPC], F32, tag="ps",
                                 name=f"pj{tp}_{ns}")
                    for cc in range(8):
                        nc.tensor.matmul(pj[:],
                                         xt[:, cc, 128 * ns:128 * (ns + 1)],
                                         wqkv_sb[:, cc, :],
                                         start=(cc == 0), stop=(cc == 7))
                    if tp <= 1:
                        nc.scalar.activation(
                            qk_nd[:, ns],
                            pj[:, 0:2 * CPC].rearrange("p (s d) -> p s d",
                                                       s=4), AF.Copy)
                        nc.scalar.activation(
                            vext[:, 8 * tp + ns, :, 0:64],
                            pj[:, 2 * CPC:3 * CPC].rearrange(
                                "p (h d) -> p h d", h=HPC), AF.Copy)
                    else:
                        nc.vector.tensor_copy(
                            qk_nd[:, ns],
                            pj[:, 0:2 * CPC].rearrange("p (s d) -> p s d",
                                                       s=4))
                        nc.vector.tensor_copy(
                            vext[:, 8 * tp + ns, :, 0:64],
                            pj[:, 2 * CPC:3 * CPC].rearrange(
                                "p (h d) -> p h d", h=HPC))
                    yield
                # LayerNorm stats over d=64 for each (row, slot)
                s1 = small.tile([128, 8, 4], F32, tag="s1", name=f"s1_{tp}")
                nc.vector.reduce_sum(s1[:], qk_nd[:], axis=AX.X)
                sq = work.tile([128, 8, 4, D], BF16, tag="tmp",
                               name=f"sq{tp}")
                if tp <= 1:
                    nc.scalar.square(sq[:], qk_nd[:])
                else:
                    # tp 2,3 run inside the attention-exp window: keep ACT
                    # free, square on DVE instead
                    nc.vector.tensor_tensor(sq[:], qk_nd[:], qk_nd[:],
                                            OP.mult)
                s2 = small.tile([128, 8, 4], F32, tag="s2", name=f"s2_{tp}")
                nc.vector.reduce_sum(s2[:], sq[:], axis=AX.X)
                mu = small.tile([128, 8, 4], F32, tag="mu", name=f"mu{tp}")
                nc.vector.tensor_scalar_mul(mu[:], s1[:], 1.0 / D)
                var = small.tile([128, 8, 4], F32, tag="var", name=f"var{tp}")
                nc.vector.tensor_scalar_mul(var[:], s2[:], 1.0 / D)
                mm = small.tile([128, 8, 4], F32, tag="mm", name=f"mm{tp}")
                nc.vector.tensor_tensor(mm[:], mu[:], mu[:], OP.mult)
                nc.vector.tensor_tensor(var[:], var[:], mm[:], OP.subtract)
                nc.vector.tensor_scalar_add(var[:], var[:], EPS)
                # rsqrt(var+eps) = exp(-0.5*ln(var+eps)) on ACT: stays in
                # the ln/exp table set (no thrash against attention's Exp)
                lnv = small.tile([128, 8, 4], F32, tag="lnv", name=f"lnv{tp}")
                nc.scalar.activation(lnv[:], var[:], AF.Ln)
                a_ = small.tile([128, 8, 4], BF16, tag="a", name=f"a{tp}")
                nc.scalar.activation(a_[:], lnv[:], AF.Exp, scale=-0.5)
                nma = small.tile([128, 8, 4], BF16, tag="nma", name=f"nma{tp}")
                nc.vector.tensor_tensor(nma[:], mu[:], a_[:], OP.mult)
                yield
                # qn = q*a - mu*a
                nc.vector.tensor_tensor(
                    qk_nd[:], qk_nd[:],
                    a_[:, :, :, None].to_broadcast((128, 8, 4, D)), OP.mult)
                nc.vector.tensor_tensor(
                    qk_nd[:], qk_nd[:],
                    nma[:, :, :, None].to_broadcast((128, 8, 4, D)),
                    OP.subtract)
                yield
                # RoPE: out = qn*cos + rot_half(qn)*sin
                cs_lo = 8 * (tp % 2)
                cos_t = cos_sb[:, cs_lo:cs_lo + 8, :]
                sin_t = sin_sb[:, cs_lo:cs_lo + 8, :]
                cs = cos_t[:, :, None, :].to_broadcast((128, 8, 4, D))
                sn0 = sin_t[:, :, None, 0:32].to_broadcast((128, 8, 4, 32))
                sn1 = sin_t[:, :, None, 32:64].to_broadcast((128, 8, 4, 32))
                tmp = work.tile([128, 8, 4, D], BF16, tag="tmp",
                                name=f"tmp{tp}")
                nc.vector.tensor_tensor(tmp[:], qk_nd[:], cs, OP.mult)
                qk_r = qkrp.tile([128, 8, 4, D], BF16, tag="qkr",
                                 name=f"qkr{tp}")
                nc.vector.tensor_tensor(qk_r[:, :, :, 0:32],
                                        qk_nd[:, :, :, 32:64], sn0, OP.mult)
                nc.vector.tensor_tensor(qk_r[:, :, :, 0:32],
                                        tmp[:, :, :, 0:32],
                                        qk_r[:, :, :, 0:32], OP.subtract)
                yield
                nc.vector.tensor_tensor(qk_r[:, :, :, 32:64],
                                        qk_nd[:, :, :, 0:32], sn1, OP.mult)
                nc.vector.tensor_tensor(qk_r[:, :, :, 32:64],
                                        tmp[:, :, :, 32:64],
                                        qk_r[:, :, :, 32:64], OP.add)
                state[tp] = qk_r
                yield

            def transpose_gen(tp):
                """PE transposes: q,k -> [ch, n].  q is split per head into
                qpad's zero-padded bands; k keeps both heads (the S matmul
                contracts K=128 against the zero padding).  For tp 0,1 the
                PSUM->SBUF copies go on ACT (idle pre-attention) so DVE's
                RoPE backlog doesn't gate the first S matmuls."""
                qk_r = state.pop(tp)
                for j in range(8):
                    g = 8 * tp + j
                    col = 128 * g
                    ptqk = ps.tile([128, 2, 128], BF16, tag="ps",
                                   name=f"ptqk{g}")
                    nc.tensor.transpose(ptqk[:, 0, :], qk_r[:, j, 0:2, :],
                                        identr[:])
                    nc.tensor.transpose(ptqk[:, 1, :], qk_r[:, j, 2:4, :],
                                        identr[:])
                    # the qpad band copies go on DVE for ALL tps: ACT
                    # activation-copies into partition sub-ranges were
                    # observed to corrupt the data (b0-only NaNs).
                    nc.vector.tensor_copy(qpad[0:64, 0, col:col + 128],
                                          ptqk[0:64, 0, :])
                    nc.vector.tensor_copy(qpad[64:128, 1, col:col + 128],
                                          ptqk[64:128, 0, :])
                    if tp <= 1:
                        nc.scalar.activation(kT[:, col:col + 128],
                                             ptqk[:, 1, :], AF.Copy)
                    else:
                        nc.vector.tensor_copy(kT[:, col:col + 128],
                                              ptqk[:, 1, :])
                    if j % 2 == 1:
                        yield

            def attn_batch(b):
                """Attention for both local heads of batch b.  Matmuls are
                emitted in same-tiling-mode runs: all four S matmuls of a
                group (row-tiled 64x128, heads interleaved so the h0/h1
                pairs execute concurrently in row groups 0-1/2-3), then
                both exps, then the previous group's four AV matmuls
                (128x128 mode, batched with the fillers that follow) --
                two PE mode switches per group instead of four."""
                col0 = N * b

                def s_block(qt, g):
                    qs = col0 + 512 * qt
                    psS = [psSp.tile([128, 2, 512], F32, tag="pss",
                                     name=f"pS{b}{h}{qt}_{g}")
                           for h in range(2)]
                    for j in range(2):
                        kc = 2 * g + j
                        for h in range(2):
                            nc.tensor.matmul(
                                psS[h][:, j, :],
                                kT[:, col0 + 128 * kc:col0 + 128 * (kc + 1)],
                                qpad[:, h, qs:qs + 512],
                                start=True, stop=True)
                    es = []
                    for h in range(2):
                        e = expp.tile([128, 2, 512], BF16, tag="es",
                                      name=f"es{b}{h}{qt}_{g}")
                        nc.scalar.activation(e[:], psS[h][:], AF.Exp,
                                             scale=0.125)
                        es.append(e)
                    return es

                def av_block(qt, g, es, pav):
                    for h in range(2):
                        for j in range(2):
                            nc.tensor.matmul(
                                pav[h][:],
                                vext[:, 16 * b + 2 * g + j, h, :],
                                es[h][:, j, :],
                                start=(g == 0 and j == 0),
                                stop=(g == 7 and j == 1))

                def norm_qt(qt, pav):
                    """softmax denominators for BOTH heads -> one 128-lane
                    broadcast + Ln + Exp, then two DVE multiplies.  The
                    broadcast reads the zero-padded den_z so it runs as a
                    K=128 matmul (no PE tiling-mode switch)."""
                    qs = col0 + 512 * qt
                    for h in range(2):
                        nc.vector.tensor_copy(den_z[0:1, h, :],
                                              pav[h][64:65, :])
                    pbc = ps.tile([128, 512], F32, tag="ps",
                                  name=f"pbc{b}{qt}")
                    for h in range(2):
                        nc.tensor.matmul(pbc[:],
                                         bones_sb[:, 128 * h:128 * (h + 1)],
                                         den_z[:, h, :],
                                         start=(h == 0), stop=(h == 1))
                    lnd = normp.tile([128, 512], F32, tag="lnd",
                                     name=f"lnd{b}{qt}")
                    nc.scalar.activation(lnd[:], pbc[:], AF.Ln)
                    bcr = normp.tile([128, 512], F32, tag="bcr",
                                     name=f"bcr{b}{qt}")
                    nc.scalar.activation(bcr[:], lnd[:], AF.Exp, scale=-1.0)
                    for h in range(2):
                        hof = D * h
                        nc.vector.tensor_tensor(
                            attn_outT[hof:hof + D, qs:qs + 512],
                            pav[h][0:64, :], bcr[hof:hof + D, :], OP.mult)

                for qt in range(4):
                    pav = [psav.tile([128, 512], F32, tag="av",
                                     name=f"pav{b}{h}{qt}")
                           for h in range(2)]
                    pend = None
                    for g in range(8):
                        es = s_block(qt, g)
                        if pend is not None:
                            av_block(qt, g - 1, pend, pav)
                        pend = es
                        if g < 7:
                            yield
                    av_block(qt, 7, pend, pav)
                    norm_qt(qt, pav)
                    yield

            # ---- AllToAll plumbing ---------------------------------------
            # chunk X covers attn_outT cols [CH0[X], CH0[X]+8*CHR[X]): dest
            # core j receives rows [CH0[X] + CHR[X]*j, +CHR[X]) -> its
            # output block X (row offset COFF[X]).  Five small collectives
            # so each hides under the next attention chunk; the last one
            # carries only 64 rows/core so its exposed tail is minimal.
            CH0 = [0, 1024, 2048, 2560, 3072, 3584]
            CHR = [128, 128, 64, 64, 64, 64]
            COFF = [0, 128, 256, 320, 384, 448]
            NCH = len(CH0)
            ccin = [dram.tile([N_CORES, 128, CHR[X]], BF16, name=f"ccin{X}")
                    for X in range(NCH)]
            ccout = [dram.tile([N_CORES, 128, CHR[X]], BF16, name=f"ccout{X}")
                     for X in range(NCH)]

            def emit_a2a(X):
                # single trigger (SP DMA triggers cost ~600ns each, serial)
                r = CHR[X]
                nc.sync.dma_start(
                    ccin[X][:].rearrange("j p n -> p j n"),
                    attn_outT[:, CH0[X]:CH0[X] + 8 * r].rearrange(
                        "p (j n) -> p j n", j=N_CORES))
                nc.gpsimd.collective_compute(
                    "AllToAll", OP.bypass,
                    replica_groups=[list(range(N_CORES))],
                    ins=[ccin[X][:].opt()], outs=[ccout[X][:].opt()])

            gat_tiles = {}

            def gat_fetch(nt):
                """gather block nt's collective output into SBUF.  Own tag
                per block (bufs=1, no slot reuse) so the DMA trigger never
                blocks the Sync queue waiting on a deferred outproj read."""
                r = CHR[nt]
                gat = freqs.tile([128, 8, r], BF16, tag=f"gat{nt}",
                                 name=f"gat{nt}", bufs=1)
                nc.sync.dma_start(gat[:],
                                  ccout[nt][:].rearrange("j p n -> p j n"))
                gat_tiles[nt] = gat

            def outproj_gen(nt):
                """project this core's CHR[nt]-row output block nt
                (gat_fetch(nt) must have been emitted already)."""
                r = CHR[nt]
                gat = gat_tiles.pop(nt)
                ob = work.tile([128, C], F32, tag="ob", name=f"ob{nt}")
                for hf in range(2):
                    po = ps.tile([128, 512], F32, tag="ps",
                                 name=f"po{nt}_{hf}")
                    for cc in range(8):
                        nc.tensor.matmul(
                            po[0:r, :],
                            gat[:, cc, :],
                            wp_sb[:, cc, 512 * hf:512 * (hf + 1)],
                            start=(cc == 0), stop=(cc == 7))
                        if cc == 3:
                            yield
                    nc.vector.tensor_tensor(
                        ob[0:r, 512 * hf:512 * (hf + 1)], po[0:r, :],
                        bias_sb[0:r, 512 * hf:512 * (hf + 1)], OP.add)
                    yield
                nc.sync.dma_start(
                    out_d.ap()[COFF[nt]:COFF[nt] + r, :], ob[0:r, :])
                yield

            def run_all(gen):
                for _ in gen:
                    pass

            def mix_steps(gen, fillers, steps, fill_per_step):
                """advance gen by `steps` yields, taking up to
                fill_per_step filler yields after each."""
                for _ in range(steps):
                    try:
                        next(gen)
                    except StopIteration:
                        break
                    took = 0
                    while fillers and took < fill_per_step:
                        try:
                            next(fillers[0])
                            took += 1
                        except StopIteration:
                            fillers.pop(0)

            def adv(gen, steps):
                for _ in range(steps):
                    try:
                        next(gen)
                    except StopIteration:
                        break

            # ---- emission schedule ---------------------------------------
            # cos/sin first: they're small (512KB) and tp0's RoPE (~40us)
            # must not race their arrival behind the multi-MB x/w stream.
            nc.sync.dma_start(cos_sb[:], cos_r[:])
            nc.sync.dma_start(sin_sb[:], sin_r[:])
            run_all(pre_gen(0, load_xt(0)))
            emit_late_consts()
            xt1 = load_xt(1)
            # xt2's triggers go out right after xt1's (xload bufs=3: no WAR
            # wait can block the Sync queue here) so its data lands before
            # the tp2 bridge blocks below need it.
            xt2 = load_xt(2)
            run_all(pre_gen(1, xt1))
            xt3 = load_xt(3)
            run_all(transpose_gen(0))
            # bridge the pre->attention PE hole (tp1 RoPE tail on DVE
            # leaves the PE idle >3.4us otherwise -> HAM re-throttles right
            # as attention starts): give the PE tp2's first QKV blocks,
            # AFTER tp0's transposes so they aren't head-of-line blocked.
            p2 = pre_gen(2, xt2)
            adv(p2, 4)
            run_all(transpose_gen(1))
            # bias only feeds the tail outproj; keep it out of the early
            # DMA stream.
            nc.sync.dma_start(bias_sb[:], bias_d.ap())
            nc.sync.dma_start(wp_sb[:],
                              wpT_d.ap().rearrange("(co p) k -> p co k", p=128))
            # batch-0 attention yields after every 2-chunk group (8 per qt,
            # 32 total); one filler step per yield keeps the PE's in-order
            # queue dense inside each ACT-paced qt (HAM stays un-throttled).
            fillers = [p2, pre_gen(3, xt3),
                       transpose_gen(2), transpose_gen(3)]
            g0 = attn_batch(0)
            mix_steps(g0, fillers, 16, 1)       # b0 qt0,qt1
            emit_a2a(0)
            mix_steps(g0, fillers, 16, 1)       # b0 qt2,qt3
            for f in fillers:
                run_all(f)
            emit_a2a(1)
            # batch-1 attention: per-qt collectives.  All outproj matmul
            # work is deferred to the tail so it hides the final AllToAll's
            # ~20us latency; only the cheap gat DMA triggers are emitted as
            # soon as each chunk's collective result is needed-by-able, and
            # every gat trigger lands BEFORE the next collective emission
            # (a later emission waits on the shared collective-output
            # semaphore and would serialize on it).
            # each gat trigger is emitted right after its own collective's
            # emission window (a trigger emitted after LATER collectives
            # waits on their completions too -- and emitting it long after
            # its collective was observed to corrupt the gather, so keep
            # trigger emission adjacent to its collective).
            gat_fetch(0)
            emit_a2a(1)
            # outproj blocks 0-2 run as b1 fillers (PE density: micro-idle
            # groups re-throttle HAM to half clock otherwise); each block
            # starts a few steps after its gat fetch so the fills don't
            # stall.  Blocks 3-5 run at the tail, 3/4 padding the final
            # AllToAll.
            g1 = attn_batch(1)
            opj = {}
            fills = {2: 0, 3: 0, 4: 0, 5: 0, 6: 0,
                     10: 1, 11: 1, 12: 1, 13: 1, 14: 1,
                     18: 2, 19: 2, 20: 2, 21: 2, 22: 2}
            for step in range(32):
                try:
                    next(g1)
                except StopIteration:
                    break
                if step == 1:
                    gat_fetch(1)
                elif step == 7:
                    emit_a2a(2)                 # b1 qt0 rows
                elif step == 9:
                    gat_fetch(2)
                elif step == 15:
                    emit_a2a(3)                 # b1 qt1 rows
                elif step == 17:
                    gat_fetch(3)
                elif step == 23:
                    emit_a2a(4)                 # b1 qt2 rows
                elif step == 25:
                    gat_fetch(4)
                X = fills.get(step)
                if X is not None:
                    if X not in opj:
                        opj[X] = outproj_gen(X)
                    adv(opj[X], 1)
            for X in range(3):
                run_all(opj[X])
            if _DBG:
                nc.sync.dma_start(dbg_ao_d.ap(), attn_outT[:])
                nc.sync.dma_start(dbg_qp_d.ap(), qpad[:])
                nc.sync.dma_start(dbg_kt_d.ap(), kT[:])
                nc.sync.dma_start(dbg_vx_d.ap(), vext[:])
            emit_a2a(5)                         # b1 qt3 rows
            run_all(outproj_gen(3))             # pad the final AllToAll
            run_all(outproj_gen(4))
            gat_fetch(5)
            run_all(outproj_gen(5))             # exposed: only 64 rows
    _split_excess_waits(nc)
    return nc


_NC_CACHE = {}


def _get_nc():
    if "nc" not in _NC_CACHE:
        _NC_CACHE["nc"] = build()
    return _NC_CACHE["nc"]


def _prep_inputs(x, w_qkv, w_proj, b_proj, freqs_cos, freqs_sin):
    x = np.asarray(x, dtype=np.float32)
    w_qkv = np.asarray(w_qkv, dtype=np.float32)
    w_proj = np.asarray(w_proj, dtype=np.float32)
    b_proj = np.asarray(b_proj, dtype=np.float32)
    bf = ml_dtypes.bfloat16
    cos = np.asarray(freqs_cos, dtype=np.float32).reshape(N, D).astype(bf)
    sin = np.asarray(freqs_sin, dtype=np.float32).reshape(N, D).astype(bf)

    xT = np.ascontiguousarray(x.reshape(NTOT, C).T).astype(bf)
    wpT = np.ascontiguousarray(w_proj.T).astype(bf)
    biasb = np.ascontiguousarray(
        np.broadcast_to(b_proj, (128, C))).astype(np.float32)
    ident = np.eye(128, dtype=np.float32)
    bones = np.zeros((128, 256), dtype=np.float32)
    bones[0, 0:64] = 1.0        # head0 lhsT: ones in cols 0-63
    bones[0, 192:256] = 1.0     # head1 lhsT: ones in cols 64-127
    bones = bones.astype(bf)

    in_maps = []
    for i in range(N_CORES):
        r0 = CPC * i
        wqkv = np.concatenate([w_qkv[r0:r0 + CPC],
                               w_qkv[C + r0:C + r0 + CPC],
                               w_qkv[2 * C + r0:2 * C + r0 + CPC]], axis=0)
        wqkvT = np.ascontiguousarray(wqkv.T).astype(bf)
        in_maps.append({
            "xT": xT, "wqkvT": wqkvT, "wpT": wpT,
            "biasb": biasb, "cosd": cos, "sind": sin, "identd": ident,
            "bonesd": bones,
        })
    return in_maps


def kernel(x, w_qkv, w_proj, b_proj, freqs_cos, freqs_sin):
    in_maps = _prep_inputs(x, w_qkv, w_proj, b_proj, freqs_cos, freqs_sin)
    nc = _get_nc()
    res = run_bass_kernel_spmd(nc, in_maps, core_ids=list(range(N_CORES)))
    CH0 = [0, 1024, 2048, 2560, 3072, 3584]
    CHR = [128, 128, 64, 64, 64, 64]
    COFF = [0, 128, 256, 320, 384, 448]
    full = np.empty((NTOT, C), dtype=np.float32)
    for i in range(N_CORES):
        o = res.results[i]["out"]
        for X in range(len(CH0)):
            r0 = CH0[X] + CHR[X] * i
            full[r0:r0 + CHR[X]] = o[COFF[X]:COFF[X] + CHR[X]]
    return full.reshape(B, N, C).astype(np.float32)



# revision 42
# speedup vs baseline: 1.0466x; 1.0466x over previous
"""Trainium2 8-core attention kernel (nn_Attention_19954418057485).

Sharding: heads are split across the 8 cores (2 heads = 128 channels
each); every core processes both batch elements for its heads.  Four
AllToAlls over all 8 cores (one per 1024-row chunk, each overlapped
with compute) swap the channel axis for the row axis, so each core
finishes the full output projection for 512 rows (4 x 128, one 128-row
block per chunk) of the flattened (B*N, C) output.

Per-core pipeline (matmuls on PE in bf16, exp on ACT, elementwise DVE):
  x^T (bf16)  --PE-->  q,k (rows,ch) + v^T        [QKV projection]
  q,k: LayerNorm (d=64) + RoPE (bf16 DVE ops), then PE transposes to
  q^T,k^T [ch, n]; v^T -> V [n, ch] with a ones column appended.
  per (batch, head): S^T = K Q^T, exp(S/8) on ACT (no max-subtraction
  needed: layernormed q,k bound |scores| <= 8), AV accumulates
  V_ext^T @ expS^T giving out^T rows 0..63 plus the softmax denominator
  in row 64 (from the ones column).  Normalization per (batch, qt):
  both heads' denominator rows -> one PE block-outer-product broadcast
  [128, 512] -> 1/x via ACT exp(-ln(x)) at full 128-lane occupancy ->
  two DVE multiplies.

Instruction emission interleaves batch-0 attention (ACT-bound) with the
batch-1 preamble (DVE-bound); output-projection matmuls are only
emitted at points where their AllToAll is provably complete so the PE's
in-order queue never head-of-line blocks on a collective.
"""
import sys

if "/opt/trn_rl_repo" not in sys.path:
    sys.path.insert(0, "/opt/trn_rl_repo")

import numpy as np
import ml_dtypes

import concourse.bass as bass
import concourse.tile as tile
from concourse import mybir
from concourse.bass_utils import run_bass_kernel_spmd

N_CORES = 8
B, N, C, H = 2, 2048, 1024, 16
D = 64
HPC = H // N_CORES          # heads per core = 2
CPC = HPC * D               # channels per core = 128
NTOT = B * N                # 4096 flattened rows
RPC = NTOT // N_CORES       # output rows per core = 512
QROW = 128                  # rows per core per collective chunk
EPS = 1e-6

BF16 = mybir.dt.bfloat16
F32 = mybir.dt.float32
AF = mybir.ActivationFunctionType
OP = mybir.AluOpType
AX = mybir.AxisListType


def _split_excess_waits(nc, max_waits=1):
    """walrus rejects instructions with more than a couple of sem-wait
    commands; split extras onto preceding same-engine NoOps."""
    for fn in nc.m.functions:
        for blk in fn.blocks:
            new_insts = []
            for ins in blk.instructions:
                si = ins.sync_info
                ow = list(si.on_wait) if si is not None and si.on_wait else []
                if len(ow) > max_waits:
                    head = ow[: len(ow) - max_waits]
                    rest = ow[len(ow) - max_waits:]
                    for i in range(0, len(head), max_waits):
                        new_insts.append(mybir.InstNoOp(
                            name=f"{ins.name}_ws{i}",
                            engine=ins.engine,
                            ins=[], outs=[],
                            sync_info=mybir.SyncInfo(
                                on_wait=head[i:i + max_waits], on_update=[]),
                        ))
                    ins.sync_info = mybir.SyncInfo(
                        on_wait=rest, on_update=list(si.on_update or []))
                new_insts.append(ins)
            blk.instructions = new_insts


def build():
    nc = bass.Bass("TRN2", target_bir_lowering=False, debug=False,
                   num_devices=N_CORES)
    xT_d = nc.dram_tensor("xT", (C, NTOT), BF16, kind="ExternalInput")
    wqkv_d = nc.dram_tensor("wqkvT", (C, 3 * CPC), BF16, kind="ExternalInput")
    wpT_d = nc.dram_tensor("wpT", (C, C), BF16, kind="ExternalInput")
    bias_d = nc.dram_tensor("biasb", (128, C), F32, kind="ExternalInput")
    # host-preswizzled to the SBUF layout [p, c, d] so the DMA runs
    # contiguous 2KB lines instead of 128B strided runs
    cos_d = nc.dram_tensor("cosd", (128, 16 * D), BF16, kind="ExternalInput")
    sin_d = nc.dram_tensor("sind", (128, 16 * D), BF16, kind="ExternalInput")
    ident_d = nc.dram_tensor("identd", (128, 128), F32, kind="ExternalInput")
    bones_d = nc.dram_tensor("bonesd", (128, 256), BF16, kind="ExternalInput")
    out_d = nc.dram_tensor("out", (RPC, C), F32, kind="ExternalOutput")
    import os as _os
    _DBG = bool(_os.environ.get("KBG_DEBUG"))
    if _DBG:
        dbg_ao_d = nc.dram_tensor("dbg_ao", (128, NTOT), BF16,
                                  kind="ExternalOutput")
        dbg_qp_d = nc.dram_tensor("dbg_qp", (128, 2, NTOT), BF16,
                                  kind="ExternalOutput")
        dbg_kt_d = nc.dram_tensor("dbg_kt", (128, NTOT), BF16,
                                  kind="ExternalOutput")
        dbg_vx_d = nc.dram_tensor("dbg_vx", (128, 32, HPC, 128), BF16,
                                  kind="ExternalOutput")

    with tile.TileContext(nc) as tc:
        with tc.tile_pool(name="consts", bufs=1) as consts, \
             tc.tile_pool(name="xload", bufs=3) as xload, \
             tc.tile_pool(name="qkrp", bufs=2) as qkrp, \
             tc.tile_pool(name="freqs", bufs=2) as freqs, \
             tc.tile_pool(name="work", bufs=3) as work, \
             tc.tile_pool(name="small", bufs=2) as small, \
             tc.tile_pool(name="exps", bufs=6) as expp, \
             tc.tile_pool(name="norm", bufs=2) as normp, \
             tc.tile_pool(name="ps", bufs=2, space="PSUM") as ps, \
             tc.tile_pool(name="psS", bufs=2, space="PSUM") as psSp, \
             tc.tile_pool(name="psav", bufs=2, space="PSUM") as psav, \
             tc.tile_pool(name="dram", bufs=1, space="DRAM") as dram:

            # ---- constants (ordered so the first QKV matmul can start
            # as early as possible: wqkv first, bias/wp deferred) ------
            wqkv_sb = consts.tile([128, 8, 3 * CPC], BF16)
            nc.sync.dma_start(wqkv_sb[:],
                              wqkv_d.ap().rearrange("(co p) k -> p co k", p=128))
            ident_f = consts.tile([128, 128], F32)
            # row 0: [0:128]=head0 column-block ones, [128:256]=head1's;
            # rows 1-127 zero so the norm broadcast matmul runs K=128
            # (same 128x128 PE tiling mode as AV/QKV -- no mode-switch
            # drain mid-attention).
            bones_sb = consts.tile([128, 256], BF16)
            # zero-padded denominator staging: row 0 of each half carries
            # the per-head softmax denominators, rows 1-127 stay zero.
            den_z = consts.tile([128, 2, 512], BF16)
            wp_sb = consts.tile([128, 8, C], BF16)      # DMA deferred
            bias_sb = consts.tile([128, C], F32)        # DMA deferred
            # all RoPE tables resident: batch 1 (tp2/3) reuses the same
            # positions as batch 0 (tp0/1), so one load serves all four tps
            cos_sb = consts.tile([128, 16, D], BF16)    # DMA deferred
            sin_sb = consts.tile([128, 16, D], BF16)    # DMA deferred
            identr = consts.tile([128, 128], BF16)
            # PE warm-up scratch: ~5us of tiny matmuls during the initial
            # DMA wait flips HAM to K=8/8 before the first real QKV matmul
            heat = consts.tile([128, 128], BF16)

            # ---- persistent tensors ---------------------------------------
            # k transposed: [ch (both heads), b*N+n]
            kT = consts.tile([128, NTOT], BF16)
            # q transposed, zero-padded per head: qpad[:, h] holds head h's
            # q rows in its 64-channel band and ZEROS in the other band, so
            # the S matmuls contract K=128 (same 128x128 PE tiling mode as
            # every other matmul -- no mode-switch drains; the padded rows
            # multiply k's other-head channels by zero).
            qpad = consts.tile([128, 2, NTOT], BF16)
            # V with ones column, padded to 128 so the AV ldweights is a
            # full-128-column load (FWL-eligible): [n%128, chunk, head,
            # 64 d + 1 one + 63 zeros]
            vext = consts.tile([128, 32, HPC, 128], BF16)
            attn_outT = consts.tile([128, NTOT], BF16)
            # zero the padded tensors FIRST (before any producer writes are
            # emitted) with simple contiguous memsets: every later write is
            # WAW-ordered after these, and strided partial memsets were
            # observed to corrupt neighbouring columns.
            nc.vector.memset(heat[:], 0.0)
            nc.vector.memset(den_z[:], 0.0)
            nc.vector.memset(qpad[:], 0.0)
            nc.vector.memset(vext[:], 0.0)

            def emit_late_consts():
                """everything not needed by the first QKV matmuls: emitted
                after pre_gen(0) so its DMA triggers don't delay xt0."""
                nc.sync.dma_start(ident_f[:], ident_d.ap())
                nc.sync.dma_start(bones_sb[:], bones_d.ap())
                nc.scalar.activation(identr[:], ident_f[:], AF.Copy)
                nc.scalar.activation(
                    vext[:, :, :, 64:65],
                    ident_f[:, 0:64].rearrange("p (a b c) -> p a b c",
                                               a=32, b=2),
                    AF.Identity, scale=0.0, bias=1.0)

            xT_r = xT_d.ap().rearrange("(co p) n -> p co n", p=128)
            state = {}

            def load_xt(tp):
                """trigger tp's x-chunk DMA (split so ns 0-3 can start
                after the first MB)."""
                xt = xload.tile([128, 8, 1024], BF16, tag="xt",
                                name=f"xt{tp}")
                nc.sync.dma_start(xt[:, :, 0:512],
                                  xT_r[:, :, 1024 * tp:1024 * tp + 512])
                nc.sync.dma_start(xt[:, :, 512:1024],
                                  xT_r[:, :, 1024 * tp + 512:1024 * (tp + 1)])
                return xt

            def pre_gen(tp, xt):
                """QKV proj + LN + RoPE for rows [tp*1024, (tp+1)*1024)."""
                qk_nd = work.tile([128, 8, 4, D], BF16, tag="qknd",
                                  name=f"qknd{tp}")
                for ns in range(8):
                    pj = ps.tile([128, 3 * CPC], F32, tag="ps",
                                 name=f"pj{tp}_{ns}")
                    for cc in range(8):
                        nc.tensor.matmul(pj[:],
                                         xt[:, cc, 128 * ns:128 * (ns + 1)],
                                         wqkv_sb[:, cc, :],
                                         start=(cc == 0), stop=(cc == 7))
                    if tp <= 1:
                        nc.scalar.activation(
                            qk_nd[:, ns],
                            pj[:, 0:2 * CPC].rearrange("p (s d) -> p s d",
                                                       s=4), AF.Copy)
                        nc.scalar.activation(
                            vext[:, 8 * tp + ns, :, 0:64],
                            pj[:, 2 * CPC:3 * CPC].rearrange(
                                "p (h d) -> p h d", h=HPC), AF.Copy)
                    else:
                        nc.vector.tensor_copy(
                            qk_nd[:, ns],
                            pj[:, 0:2 * CPC].rearrange("p (s d) -> p s d",
                                                       s=4))
                        nc.vector.tensor_copy(
                            vext[:, 8 * tp + ns, :, 0:64],
                            pj[:, 2 * CPC:3 * CPC].rearrange(
                                "p (h d) -> p h d", h=HPC))
                    yield
                # LayerNorm stats over d=64 for each (row, slot)
                s1 = small.tile([128, 8, 4], F32, tag="s1", name=f"s1_{tp}")
                nc.vector.reduce_sum(s1[:], qk_nd[:], axis=AX.X)
                sq = work.tile([128, 8, 4, D], BF16, tag="tmp",
                               name=f"sq{tp}")
                if tp <= 1:
                    nc.scalar.square(sq[:], qk_nd[:])
                else:
                    # tp 2,3 run inside the attention-exp window: keep ACT
                    # free, square on DVE instead
                    nc.vector.tensor_tensor(sq[:], qk_nd[:], qk_nd[:],
                                            OP.mult)
                s2 = small.tile([128, 8, 4], F32, tag="s2", name=f"s2_{tp}")
                nc.vector.reduce_sum(s2[:], sq[:], axis=AX.X)
                mu = small.tile([128, 8, 4], F32, tag="mu", name=f"mu{tp}")
                nc.vector.tensor_scalar_mul(mu[:], s1[:], 1.0 / D)
                var = small.tile([128, 8, 4], F32, tag="var", name=f"var{tp}")
                nc.vector.tensor_scalar_mul(var[:], s2[:], 1.0 / D)
                mm = small.tile([128, 8, 4], F32, tag="mm", name=f"mm{tp}")
                nc.vector.tensor_tensor(mm[:], mu[:], mu[:], OP.mult)
                nc.vector.tensor_tensor(var[:], var[:], mm[:], OP.subtract)
                nc.vector.tensor_scalar_add(var[:], var[:], EPS)
                # rsqrt(var+eps) = exp(-0.5*ln(var+eps)) on ACT: stays in
                # the ln/exp table set (no thrash against attention's Exp)
                lnv = small.tile([128, 8, 4], F32, tag="lnv", name=f"lnv{tp}")
                nc.scalar.activation(lnv[:], var[:], AF.Ln)
                a_ = small.tile([128, 8, 4], BF16, tag="a", name=f"a{tp}")
                nc.scalar.activation(a_[:], lnv[:], AF.Exp, scale=-0.5)
                nma = small.tile([128, 8, 4], BF16, tag="nma", name=f"nma{tp}")
                nc.vector.tensor_tensor(nma[:], mu[:], a_[:], OP.mult)
                yield
                # qn = q*a - mu*a
                nc.vector.tensor_tensor(
                    qk_nd[:], qk_nd[:],
                    a_[:, :, :, None].to_broadcast((128, 8, 4, D)), OP.mult)
                nc.vector.tensor_tensor(
                    qk_nd[:], qk_nd[:],
                    nma[:, :, :, None].to_broadcast((128, 8, 4, D)),
                    OP.subtract)
                yield
                # RoPE: out = qn*cos + rot_half(qn)*sin
                cs_lo = 8 * (tp % 2)
                cos_t = cos_sb[:, cs_lo:cs_lo + 8, :]
                sin_t = sin_sb[:, cs_lo:cs_lo + 8, :]
                cs = cos_t[:, :, None, :].to_broadcast((128, 8, 4, D))
                sn0 = sin_t[:, :, None, 0:32].to_broadcast((128, 8, 4, 32))
                sn1 = sin_t[:, :, None, 32:64].to_broadcast((128, 8, 4, 32))
                tmp = work.tile([128, 8, 4, D], BF16, tag="tmp",
                                name=f"tmp{tp}")
                nc.vector.tensor_tensor(tmp[:], qk_nd[:], cs, OP.mult)
                qk_r = qkrp.tile([128, 8, 4, D], BF16, tag="qkr",
                                 name=f"qkr{tp}")
                nc.vector.tensor_tensor(qk_r[:, :, :, 0:32],
                                        qk_nd[:, :, :, 32:64], sn0, OP.mult)
                nc.vector.tensor_tensor(qk_r[:, :, :, 0:32],
                                        tmp[:, :, :, 0:32],
                                        qk_r[:, :, :, 0:32], OP.subtract)
                yield
                nc.vector.tensor_tensor(qk_r[:, :, :, 32:64],
                                        qk_nd[:, :, :, 0:32], sn1, OP.mult)
                nc.vector.tensor_tensor(qk_r[:, :, :, 32:64],
                                        tmp[:, :, :, 32:64],
                                        qk_r[:, :, :, 32:64], OP.add)
                state[tp] = qk_r
                yield

            def transpose_gen(tp):
                """PE transposes: q,k -> [ch, n].  q is split per head into
                qpad's zero-padded bands; k keeps both heads (the S matmul
                contracts K=128 against the zero padding).  For tp 0,1 the
                PSUM->SBUF copies go on ACT (idle pre-attention) so DVE's
                RoPE backlog doesn't gate the first S matmuls."""
                qk_r = state.pop(tp)
                for j in range(8):
                    g = 8 * tp + j
                    col = 128 * g
                    ptqk = ps.tile([128, 2, 128], BF16, tag="ps",
                                   name=f"ptqk{g}")
                    nc.tensor.transpose(ptqk[:, 0, :], qk_r[:, j, 0:2, :],
                                        identr[:])
                    nc.tensor.transpose(ptqk[:, 1, :], qk_r[:, j, 2:4, :],
                                        identr[:])
                    # the qpad band copies go on DVE for ALL tps: ACT
                    # activation-copies into partition sub-ranges were
                    # observed to corrupt the data (b0-only NaNs).
                    nc.vector.tensor_copy(qpad[0:64, 0, col:col + 128],
                                          ptqk[0:64, 0, :])
                    nc.vector.tensor_copy(qpad[64:128, 1, col:col + 128],
                                          ptqk[64:128, 0, :])
                    if tp <= 1:
                        nc.scalar.activation(kT[:, col:col + 128],
                                             ptqk[:, 1, :], AF.Copy)
                    else:
                        nc.vector.tensor_copy(kT[:, col:col + 128],
                                              ptqk[:, 1, :])
                    if j % 2 == 1:
                        yield

            def attn_batch(b):
                """Attention for both local heads of batch b.  Matmuls are
                emitted in same-tiling-mode runs: all four S matmuls of a
                group (row-tiled 64x128, heads interleaved so the h0/h1
                pairs execute concurrently in row groups 0-1/2-3), then
                both exps, then the previous group's four AV matmuls
                (128x128 mode, batched with the fillers that follow) --
                two PE mode switches per group instead of four."""
                col0 = N * b

                def s_block(qt, g):
                    qs = col0 + 512 * qt
                    psS = [psSp.tile([128, 2, 512], F32, tag="pss",
                                     name=f"pS{b}{h}{qt}_{g}")
                           for h in range(2)]
                    for j in range(2):
                        kc = 2 * g + j
                        for h in range(2):
                            nc.tensor.matmul(
                                psS[h][:, j, :],
                                kT[:, col0 + 128 * kc:col0 + 128 * (kc + 1)],
                                qpad[:, h, qs:qs + 512],
                                start=True, stop=True)
                    es = []
                    for h in range(2):
                        e = expp.tile([128, 2, 512], BF16, tag="es",
                                      name=f"es{b}{h}{qt}_{g}")
                        nc.scalar.activation(e[:], psS[h][:], AF.Exp,
                                             scale=0.125)
                        es.append(e)
                    return es

                def av_block(qt, g, es, pav):
                    for h in range(2):
                        for j in range(2):
                            nc.tensor.matmul(
                                pav[h][:],
                                vext[:, 16 * b + 2 * g + j, h, :],
                                es[h][:, j, :],
                                start=(g == 0 and j == 0),
                                stop=(g == 7 and j == 1))

                def norm_qt(qt, pav):
                    """softmax denominators for BOTH heads -> one 128-lane
                    broadcast + Ln + Exp, then two DVE multiplies.  The
                    broadcast reads the zero-padded den_z so it runs as a
                    K=128 matmul (no PE tiling-mode switch)."""
                    qs = col0 + 512 * qt
                    # den copies on ACT: they gate the ln/exp right after
                    # on the same queue, while DVE may be several microsecs
                    # behind on filler work (cross-engine stall otherwise)
                    for h in range(2):
                        nc.scalar.activation(den_z[0:1, h, :],
                                             pav[h][64:65, :], AF.Copy)
                    pbc = ps.tile([128, 512], F32, tag="ps",
                                  name=f"pbc{b}{qt}")
                    for h in range(2):
                        nc.tensor.matmul(pbc[:],
                                         bones_sb[:, 128 * h:128 * (h + 1)],
                                         den_z[:, h, :],
                                         start=(h == 0), stop=(h == 1))
                    lnd = normp.tile([128, 512], F32, tag="lnd",
                                     name=f"lnd{b}{qt}")
                    nc.scalar.activation(lnd[:], pbc[:], AF.Ln)
                    bcr = normp.tile([128, 512], F32, tag="bcr",
                                     name=f"bcr{b}{qt}")
                    nc.scalar.activation(bcr[:], lnd[:], AF.Exp, scale=-1.0)
                    for h in range(2):
                        hof = D * h
                        nc.vector.tensor_tensor(
                            attn_outT[hof:hof + D, qs:qs + 512],
                            pav[h][0:64, :], bcr[hof:hof + D, :], OP.mult)

                for qt in range(4):
                    pav = [psav.tile([128, 512], F32, tag="av",
                                     name=f"pav{b}{h}{qt}")
                           for h in range(2)]
                    pend = None
                    for g in range(8):
                        es = s_block(qt, g)
                        if pend is not None:
                            av_block(qt, g - 1, pend, pav)
                        pend = es
                        if g < 7:
                            yield
                    av_block(qt, 7, pend, pav)
                    norm_qt(qt, pav)
                    yield

            # ---- AllToAll plumbing ---------------------------------------
            # chunk X covers attn_outT cols [CH0[X], CH0[X]+8*CHR[X]): dest
            # core j receives rows [CH0[X] + CHR[X]*j, +CHR[X]) -> its
            # output block X (row offset COFF[X]).  Five small collectives
            # so each hides under the next attention chunk; the last one
            # carries only 64 rows/core so its exposed tail is minimal.
            CH0 = [0, 1024, 2048, 2560, 3072, 3584]
            CHR = [128, 128, 64, 64, 64, 64]
            COFF = [0, 128, 256, 320, 384, 448]
            NCH = len(CH0)
            ccin = [dram.tile([N_CORES, 128, CHR[X]], BF16, name=f"ccin{X}")
                    for X in range(NCH)]
            ccout = [dram.tile([N_CORES, 128, CHR[X]], BF16, name=f"ccout{X}")
                     for X in range(NCH)]

            def emit_a2a(X):
                # single trigger (SP DMA triggers cost ~600ns each, serial)
                r = CHR[X]
                nc.sync.dma_start(
                    ccin[X][:].rearrange("j p n -> p j n"),
                    attn_outT[:, CH0[X]:CH0[X] + 8 * r].rearrange(
                        "p (j n) -> p j n", j=N_CORES))
                nc.gpsimd.collective_compute(
                    "AllToAll", OP.bypass,
                    replica_groups=[list(range(N_CORES))],
                    ins=[ccin[X][:].opt()], outs=[ccout[X][:].opt()])

            gat_tiles = {}

            def gat_fetch(nt):
                """gather block nt's collective output into SBUF.  Own tag
                per block (bufs=1, no slot reuse) so the DMA trigger never
                blocks the Sync queue waiting on a deferred outproj read."""
                r = CHR[nt]
                gat = freqs.tile([128, 8, r], BF16, tag=f"gat{nt}",
                                 name=f"gat{nt}", bufs=1)
                nc.sync.dma_start(gat[:],
                                  ccout[nt][:].rearrange("j p n -> p j n"))
                gat_tiles[nt] = gat

            def outproj_gen(nt):
                """project this core's CHR[nt]-row output block nt
                (gat_fetch(nt) must have been emitted already)."""
                r = CHR[nt]
                gat = gat_tiles.pop(nt)
                ob = work.tile([128, C], F32, tag="ob", name=f"ob{nt}")
                for hf in range(2):
                    po = ps.tile([128, 512], F32, tag="ps",
                                 name=f"po{nt}_{hf}")
                    for cc in range(8):
                        nc.tensor.matmul(
                            po[0:r, :],
                            gat[:, cc, :],
                            wp_sb[:, cc, 512 * hf:512 * (hf + 1)],
                            start=(cc == 0), stop=(cc == 7))
                        if cc == 3:
                            yield
                    nc.vector.tensor_tensor(
                        ob[0:r, 512 * hf:512 * (hf + 1)], po[0:r, :],
                        bias_sb[0:r, 512 * hf:512 * (hf + 1)], OP.add)
                    yield
                nc.sync.dma_start(
                    out_d.ap()[COFF[nt]:COFF[nt] + r, :], ob[0:r, :])
                yield

            def run_all(gen):
                for _ in gen:
                    pass

            def mix_steps(gen, fillers, steps, fill_per_step):
                """advance gen by `steps` yields, taking up to
                fill_per_step filler yields after each."""
                for _ in range(steps):
                    try:
                        next(gen)
                    except StopIteration:
                        break
                    took = 0
                    while fillers and took < fill_per_step:
                        try:
                            next(fillers[0])
                            took += 1
                        except StopIteration:
                            fillers.pop(0)

            def adv(gen, steps):
                for _ in range(steps):
                    try:
                        next(gen)
                    except StopIteration:
                        break

            # ---- emission schedule ---------------------------------------
            # PE heater: ~96 tiny matmuls fill the initial DMA wait and trip
            # HAM to full clock before the first real QKV matmul.
            pheat = ps.tile([128, 64], F32, tag="ps", name="pheat")
            for _ in range(96):
                nc.tensor.matmul(pheat[:], heat[:], heat[:, 0:64],
                                 start=True, stop=True)
            xt0 = load_xt(0)
            # cos/sin after xt0 (needed from ~38us; contiguous + small)
            nc.sync.dma_start(cos_sb[:], cos_d.ap().rearrange(
                "p (c d) -> p c d", d=D))
            nc.sync.dma_start(sin_sb[:], sin_d.ap().rearrange(
                "p (c d) -> p c d", d=D))
            run_all(pre_gen(0, xt0))
            emit_late_consts()
            xt1 = load_xt(1)
            # xt2's triggers go out right after xt1's (xload bufs=3: no WAR
            # wait can block the Sync queue here) so its data lands before
            # the tp2 bridge blocks below need it.
            xt2 = load_xt(2)
            run_all(pre_gen(1, xt1))
            xt3 = load_xt(3)
            run_all(transpose_gen(0))
            # bridge the pre->attention PE hole (tp1 RoPE tail on DVE
            # leaves the PE idle >3.4us otherwise -> HAM re-throttles right
            # as attention starts): give the PE tp2's first QKV blocks,
            # AFTER tp0's transposes so they aren't head-of-line blocked.
            p2 = pre_gen(2, xt2)
            adv(p2, 4)
            run_all(transpose_gen(1))
            # bias only feeds the tail outproj; keep it out of the early
            # DMA stream.
            nc.sync.dma_start(bias_sb[:], bias_d.ap())
            nc.sync.dma_start(wp_sb[:],
                              wpT_d.ap().rearrange("(co p) k -> p co k", p=128))
            # batch-0 attention yields after every 2-chunk group (8 per qt,
            # 32 total); one filler step per yield keeps the PE's in-order
            # queue dense inside each ACT-paced qt (HAM stays un-throttled).
            fillers = [p2, pre_gen(3, xt3),
                       transpose_gen(2), transpose_gen(3)]
            g0 = attn_batch(0)
            mix_steps(g0, fillers, 16, 1)       # b0 qt0,qt1
            emit_a2a(0)
            mix_steps(g0, fillers, 16, 1)       # b0 qt2,qt3
            for f in fillers:
                run_all(f)
            emit_a2a(1)
            # batch-1 attention: per-qt collectives.  All outproj matmul
            # work is deferred to the tail so it hides the final AllToAll's
            # ~20us latency; only the cheap gat DMA triggers are emitted as
            # soon as each chunk's collective result is needed-by-able, and
            # every gat trigger lands BEFORE the next collective emission
            # (a later emission waits on the shared collective-output
            # semaphore and would serialize on it).
            # each gat trigger is emitted right after its own collective's
            # emission window (a trigger emitted after LATER collectives
            # waits on their completions too -- and emitting it long after
            # its collective was observed to corrupt the gather, so keep
            # trigger emission adjacent to its collective).
            gat_fetch(0)
            emit_a2a(1)
            # outproj blocks 0-2 run as b1 fillers (PE density: micro-idle
            # groups re-throttle HAM to half clock otherwise); each block
            # starts a few steps after its gat fetch so the fills don't
            # stall.  Blocks 3-5 run at the tail, 3/4 padding the final
            # AllToAll.
            g1 = attn_batch(1)
            opj = {}
            # fills start one qt after each gat fetch: a2a0 can complete as
            # late as ~190us, so opj0 must not head-of-line block the PE
            # queue at b1's first qt.
            fills = {10: 0, 11: 0, 12: 0, 13: 0, 14: 0,
                     18: 1, 19: 1, 20: 1, 21: 1, 22: 1,
                     26: 2, 27: 2, 28: 2, 29: 2, 30: 2}
            for step in range(32):
                try:
                    next(g1)
                except StopIteration:
                    break
                if step == 1:
                    gat_fetch(1)
                elif step == 7:
                    emit_a2a(2)                 # b1 qt0 rows
                elif step == 9:
                    gat_fetch(2)
                elif step == 15:
                    emit_a2a(3)                 # b1 qt1 rows
                elif step == 17:
                    gat_fetch(3)
                elif step == 23:
                    emit_a2a(4)                 # b1 qt2 rows
                elif step == 25:
                    gat_fetch(4)
                X = fills.get(step)
                if X is not None:
                    if X not in opj:
                        opj[X] = outproj_gen(X)
                    adv(opj[X], 1)
            for X in range(3):
                run_all(opj[X])
            if _DBG:
                nc.sync.dma_start(dbg_ao_d.ap(), attn_outT[:])
                nc.sync.dma_start(dbg_qp_d.ap(), qpad[:])
                nc.sync.dma_start(dbg_kt_d.ap(), kT[:])
                nc.sync.dma_start(dbg_vx_d.ap(), vext[:])
            emit_a2a(5)                         # b1 qt3 rows
            run_all(outproj_gen(3))             # pad the final AllToAll
            run_all(outproj_gen(4))
            gat_fetch(5)
            run_all(outproj_gen(5))             # exposed: only 64 rows
    _split_excess_waits(nc)
    return nc


_NC_CACHE = {}


def _get_nc():
    if "nc" not in _NC_CACHE:
        _NC_CACHE["nc"] = build()
    return _NC_CACHE["nc"]


def _prep_inputs(x, w_qkv, w_proj, b_proj, freqs_cos, freqs_sin):
    x = np.asarray(x, dtype=np.float32)
    w_qkv = np.asarray(w_qkv, dtype=np.float32)
    w_proj = np.asarray(w_proj, dtype=np.float32)
    b_proj = np.asarray(b_proj, dtype=np.float32)
    bf = ml_dtypes.bfloat16

    def _swz(t):
        # [N, D] -> [p, c*D] matching the SBUF tile layout (n = c*128 + p)
        t = np.asarray(t, dtype=np.float32).reshape(16, 128, D)
        return np.ascontiguousarray(
            t.transpose(1, 0, 2).reshape(128, 16 * D)).astype(bf)

    cos = _swz(freqs_cos)
    sin = _swz(freqs_sin)

    xT = np.ascontiguousarray(x.reshape(NTOT, C).T).astype(bf)
    wpT = np.ascontiguousarray(w_proj.T).astype(bf)
    biasb = np.ascontiguousarray(
        np.broadcast_to(b_proj, (128, C))).astype(np.float32)
    ident = np.eye(128, dtype=np.float32)
    bones = np.zeros((128, 256), dtype=np.float32)
    bones[0, 0:64] = 1.0        # head0 lhsT: ones in cols 0-63
    bones[0, 192:256] = 1.0     # head1 lhsT: ones in cols 64-127
    bones = bones.astype(bf)

    in_maps = []
    for i in range(N_CORES):
        r0 = CPC * i
        wqkv = np.concatenate([w_qkv[r0:r0 + CPC],
                               w_qkv[C + r0:C + r0 + CPC],
                               w_qkv[2 * C + r0:2 * C + r0 + CPC]], axis=0)
        wqkvT = np.ascontiguousarray(wqkv.T).astype(bf)
        in_maps.append({
            "xT": xT, "wqkvT": wqkvT, "wpT": wpT,
            "biasb": biasb, "cosd": cos, "sind": sin, "identd": ident,
            "bonesd": bones,
        })
    return in_maps


def kernel(x, w_qkv, w_proj, b_proj, freqs_cos, freqs_sin):
    in_maps = _prep_inputs(x, w_qkv, w_proj, b_proj, freqs_cos, freqs_sin)
    nc = _get_nc()
    res = run_bass_kernel_spmd(nc, in_maps, core_ids=list(range(N_CORES)))
    CH0 = [0, 1024, 2048, 2560, 3072, 3584]
    CHR = [128, 128, 64, 64, 64, 64]
    COFF = [0, 128, 256, 320, 384, 448]
    full = np.empty((NTOT, C), dtype=np.float32)
    for i in range(N_CORES):
        o = res.results[i]["out"]
        for X in range(len(CH0)):
            r0 = CH0[X] + CHR[X] * i
            full[r0:r0 + CHR[X]] = o[COFF[X]:COFF[X] + CHR[X]]
    return full.reshape(B, N, C).astype(np.float32)



# revision 54
# speedup vs baseline: 1.0773x; 1.0293x over previous
"""Trainium2 8-core attention kernel (nn_Attention_19954418057485).

Sharding: heads are split across the 8 cores (2 heads = 128 channels
each); every core processes both batch elements for its heads.  Six
AllToAlls over all 8 cores (one per completed output-row chunk, each
overlapped with compute) swap the channel axis for the row axis, so
each core finishes the full output projection for 512 rows of the
flattened (B*N, C) output.

Per-core pipeline (matmuls on PE in bf16, exp on ACT, elementwise DVE):
  x^T (bf16)  --PE-->  q,k (rows,ch) + v^T        [QKV projection]
  q,k: LayerNorm (d=64) + RoPE (bf16 DVE ops), then PE transposes to
  [ch, n]; q lands in per-head zero-padded bands (qpad) so the S
  matmuls contract K=128 and EVERY matmul in the kernel runs in the
  same 128x128 PE tiling mode (no array-drain mode switches; row-tiled
  K=64 concurrency was tried and does not engage on this toolchain).
  v -> V [n, ch] with a ones column at index 64, zero-padded to 128
  columns so the AV weight loads are full-width (FWL-eligible).
  per (batch, head): S^T = K_full^T-block @ qpad, exp(S/8) on ACT (no
  max-subtraction: layernormed q,k bound the scores), AV accumulates
  V_ext^T @ expS^T giving out^T rows 0..63 plus the softmax denominator
  in row 64.  Normalization per (batch, qt): denominator rows -> one
  K=128 PE broadcast matmul via a zero-padded ones matrix (still
  128x128 mode) -> 1/x via ACT exp(-ln(x)) -> two DVE multiplies.

Schedule: ~96 tiny heater matmuls trip the PE's HAM clock-gate to full
rate during the initial DMA wait; cos/sin arrive host-preswizzled
(contiguous DMA) ahead of the bulk x stream; tp2's first QKV blocks
bridge the PE hole while tp0/1's RoPE drains on DVE; batch-0 attention
interleaves the batch-1 preamble as fillers; output projection runs as
batch-1 fillers one qt after each chunk's AllToAll completes, with the
last two blocks padding the final AllToAll's exposed latency.
"""
import sys

if "/opt/trn_rl_repo" not in sys.path:
    sys.path.insert(0, "/opt/trn_rl_repo")

import numpy as np
import ml_dtypes

import concourse.bass as bass
import concourse.tile as tile
from concourse import mybir
from concourse.bass_utils import run_bass_kernel_spmd

N_CORES = 8
B, N, C, H = 2, 2048, 1024, 16
D = 64
HPC = H // N_CORES          # heads per core = 2
CPC = HPC * D               # channels per core = 128
NTOT = B * N                # 4096 flattened rows
RPC = NTOT // N_CORES       # output rows per core = 512
QROW = 128                  # rows per core per collective chunk
EPS = 1e-6

BF16 = mybir.dt.bfloat16
F32 = mybir.dt.float32
AF = mybir.ActivationFunctionType
OP = mybir.AluOpType
AX = mybir.AxisListType


def _split_excess_waits(nc, max_waits=1):
    """walrus rejects instructions with more than a couple of sem-wait
    commands; split extras onto preceding same-engine NoOps."""
    for fn in nc.m.functions:
        for blk in fn.blocks:
            new_insts = []
            for ins in blk.instructions:
                si = ins.sync_info
                ow = list(si.on_wait) if si is not None and si.on_wait else []
                if len(ow) > max_waits:
                    head = ow[: len(ow) - max_waits]
                    rest = ow[len(ow) - max_waits:]
                    for i in range(0, len(head), max_waits):
                        new_insts.append(mybir.InstNoOp(
                            name=f"{ins.name}_ws{i}",
                            engine=ins.engine,
                            ins=[], outs=[],
                            sync_info=mybir.SyncInfo(
                                on_wait=head[i:i + max_waits], on_update=[]),
                        ))
                    ins.sync_info = mybir.SyncInfo(
                        on_wait=rest, on_update=list(si.on_update or []))
                new_insts.append(ins)
            blk.instructions = new_insts


def build():
    nc = bass.Bass("TRN2", target_bir_lowering=False, debug=False,
                   num_devices=N_CORES)
    xT_d = nc.dram_tensor("xT", (C, NTOT), BF16, kind="ExternalInput")
    wqkv_d = nc.dram_tensor("wqkvT", (C, 3 * CPC), BF16, kind="ExternalInput")
    wpT_d = nc.dram_tensor("wpT", (C, C), BF16, kind="ExternalInput")
    bias_d = nc.dram_tensor("biasb", (128, C), F32, kind="ExternalInput")
    # host-preswizzled to the SBUF layout [p, c, d] so the DMA runs
    # contiguous 2KB lines instead of 128B strided runs
    cos_d = nc.dram_tensor("cosd", (128, 16 * D), BF16, kind="ExternalInput")
    sin_d = nc.dram_tensor("sind", (128, 16 * D), BF16, kind="ExternalInput")
    ident_d = nc.dram_tensor("identd", (128, 128), F32, kind="ExternalInput")
    bones_d = nc.dram_tensor("bonesd", (128, 256), BF16, kind="ExternalInput")
    out_d = nc.dram_tensor("out", (RPC, C), F32, kind="ExternalOutput")
    import os as _os
    _DBG = bool(_os.environ.get("KBG_DEBUG"))
    if _DBG:
        dbg_ao_d = nc.dram_tensor("dbg_ao", (128, NTOT), BF16,
                                  kind="ExternalOutput")
        dbg_qp_d = nc.dram_tensor("dbg_qp", (128, 2, NTOT), BF16,
                                  kind="ExternalOutput")
        dbg_kt_d = nc.dram_tensor("dbg_kt", (128, NTOT), BF16,
                                  kind="ExternalOutput")
        dbg_vx_d = nc.dram_tensor("dbg_vx", (128, 32, HPC, 128), BF16,
                                  kind="ExternalOutput")

    with tile.TileContext(nc) as tc:
        with tc.tile_pool(name="consts", bufs=1) as consts, \
             tc.tile_pool(name="xload", bufs=3) as xload, \
             tc.tile_pool(name="qkrp", bufs=2) as qkrp, \
             tc.tile_pool(name="freqs", bufs=2) as freqs, \
             tc.tile_pool(name="work", bufs=3) as work, \
             tc.tile_pool(name="small", bufs=2) as small, \
             tc.tile_pool(name="exps", bufs=6) as expp, \
             tc.tile_pool(name="norm", bufs=2) as normp, \
             tc.tile_pool(name="ps", bufs=2, space="PSUM") as ps, \
             tc.tile_pool(name="psS", bufs=2, space="PSUM") as psSp, \
             tc.tile_pool(name="psav", bufs=2, space="PSUM") as psav, \
             tc.tile_pool(name="dram", bufs=1, space="DRAM") as dram:

            # ---- constants (ordered so the first QKV matmul can start
            # as early as possible: wqkv first, bias/wp deferred) ------
            wqkv_sb = consts.tile([128, 8, 3 * CPC], BF16)
            nc.sync.dma_start(wqkv_sb[:],
                              wqkv_d.ap().rearrange("(co p) k -> p co k", p=128))
            ident_f = consts.tile([128, 128], F32)
            # row 0: [0:128]=head0 column-block ones, [128:256]=head1's;
            # rows 1-127 zero so the norm broadcast matmul runs K=128
            # (same 128x128 PE tiling mode as AV/QKV -- no mode-switch
            # drain mid-attention).
            bones_sb = consts.tile([128, 256], BF16)
            # zero-padded denominator staging: row 0 of each half carries
            # the per-head softmax denominators, rows 1-127 stay zero.
            den_z = consts.tile([128, 2, 512], BF16)
            wp_sb = consts.tile([128, 8, C], BF16)      # DMA deferred
            bias_sb = consts.tile([128, C], F32)        # DMA deferred
            # all RoPE tables resident: batch 1 (tp2/3) reuses the same
            # positions as batch 0 (tp0/1), so one load serves all four tps
            cos_sb = consts.tile([128, 16, D], BF16)    # DMA deferred
            sin_sb = consts.tile([128, 16, D], BF16)    # DMA deferred
            identr = consts.tile([128, 128], BF16)
            # PE warm-up scratch: ~5us of tiny matmuls during the initial
            # DMA wait flips HAM to K=8/8 before the first real QKV matmul
            heat = consts.tile([128, 128], BF16)

            # ---- persistent tensors ---------------------------------------
            # k transposed: [ch (both heads), b*N+n]
            kT = consts.tile([128, NTOT], BF16)
            # q transposed, zero-padded per head: qpad[:, h] holds head h's
            # q rows in its 64-channel band and ZEROS in the other band, so
            # the S matmuls contract K=128 (same 128x128 PE tiling mode as
            # every other matmul -- no mode-switch drains; the padded rows
            # multiply k's other-head channels by zero).
            qpad = consts.tile([128, 2, NTOT], BF16)
            # V with ones column, padded to 128 so the AV ldweights is a
            # full-128-column load (FWL-eligible): [n%128, chunk, head,
            # 64 d + 1 one + 63 zeros]
            vext = consts.tile([128, 32, HPC, 128], BF16)
            attn_outT = consts.tile([128, NTOT], BF16)
            # zero the padded tensors FIRST (before any producer writes are
            # emitted) with simple contiguous memsets: every later write is
            # WAW-ordered after these, and strided partial memsets were
            # observed to corrupt neighbouring columns.
            nc.vector.memset(heat[:], 0.0)
            nc.vector.memset(den_z[:], 0.0)
            nc.vector.memset(qpad[:], 0.0)
            nc.vector.memset(vext[:], 0.0)

            def emit_late_consts():
                """everything not needed by the first QKV matmuls: emitted
                after pre_gen(0) so its DMA triggers don't delay xt0."""
                nc.sync.dma_start(ident_f[:], ident_d.ap())
                nc.sync.dma_start(bones_sb[:], bones_d.ap())
                nc.scalar.activation(identr[:], ident_f[:], AF.Copy)
                nc.scalar.activation(
                    vext[:, :, :, 64:65],
                    ident_f[:, 0:64].rearrange("p (a b c) -> p a b c",
                                               a=32, b=2),
                    AF.Identity, scale=0.0, bias=1.0)

            xT_r = xT_d.ap().rearrange("(co p) n -> p co n", p=128)
            state = {}

            def load_xt(tp):
                """trigger tp's x-chunk DMA (split so ns 0-3 can start
                after the first MB)."""
                xt = xload.tile([128, 8, 1024], BF16, tag="xt",
                                name=f"xt{tp}")
                nc.sync.dma_start(xt[:, :, 0:512],
                                  xT_r[:, :, 1024 * tp:1024 * tp + 512])
                nc.sync.dma_start(xt[:, :, 512:1024],
                                  xT_r[:, :, 1024 * tp + 512:1024 * (tp + 1)])
                return xt

            def pre_gen(tp, xt):
                """QKV proj + LN + RoPE for rows [tp*1024, (tp+1)*1024)."""
                qk_nd = work.tile([128, 8, 4, D], BF16, tag="qknd",
                                  name=f"qknd{tp}")
                for ns in range(8):
                    pj = ps.tile([128, 3 * CPC], F32, tag="ps",
                                 name=f"pj{tp}_{ns}")
                    for cc in range(8):
                        nc.tensor.matmul(pj[:],
                                         xt[:, cc, 128 * ns:128 * (ns + 1)],
                                         wqkv_sb[:, cc, :],
                                         start=(cc == 0), stop=(cc == 7))
                    if tp <= 1:
                        nc.scalar.activation(
                            qk_nd[:, ns],
                            pj[:, 0:2 * CPC].rearrange("p (s d) -> p s d",
                                                       s=4), AF.Copy)
                        nc.scalar.activation(
                            vext[:, 8 * tp + ns, :, 0:64],
                            pj[:, 2 * CPC:3 * CPC].rearrange(
                                "p (h d) -> p h d", h=HPC), AF.Copy)
                    else:
                        nc.vector.tensor_copy(
                            qk_nd[:, ns],
                            pj[:, 0:2 * CPC].rearrange("p (s d) -> p s d",
                                                       s=4))
                        nc.vector.tensor_copy(
                            vext[:, 8 * tp + ns, :, 0:64],
                            pj[:, 2 * CPC:3 * CPC].rearrange(
                                "p (h d) -> p h d", h=HPC))
                    yield
                # LayerNorm stats over d=64 for each (row, slot)
                s1 = small.tile([128, 8, 4], F32, tag="s1", name=f"s1_{tp}")
                nc.vector.reduce_sum(s1[:], qk_nd[:], axis=AX.X)
                sq = work.tile([128, 8, 4, D], BF16, tag="tmp",
                               name=f"sq{tp}")
                if tp <= 1:
                    nc.scalar.square(sq[:], qk_nd[:])
                else:
                    # tp 2,3 run inside the attention-exp window: keep ACT
                    # free, square on DVE instead
                    nc.vector.tensor_tensor(sq[:], qk_nd[:], qk_nd[:],
                                            OP.mult)
                s2 = small.tile([128, 8, 4], F32, tag="s2", name=f"s2_{tp}")
                nc.vector.reduce_sum(s2[:], sq[:], axis=AX.X)
                mu = small.tile([128, 8, 4], F32, tag="mu", name=f"mu{tp}")
                nc.vector.tensor_scalar_mul(mu[:], s1[:], 1.0 / D)
                var = small.tile([128, 8, 4], F32, tag="var", name=f"var{tp}")
                nc.vector.tensor_scalar_mul(var[:], s2[:], 1.0 / D)
                mm = small.tile([128, 8, 4], F32, tag="mm", name=f"mm{tp}")
                nc.vector.tensor_tensor(mm[:], mu[:], mu[:], OP.mult)
                nc.vector.tensor_tensor(var[:], var[:], mm[:], OP.subtract)
                nc.vector.tensor_scalar_add(var[:], var[:], EPS)
                # rsqrt(var+eps) = exp(-0.5*ln(var+eps)) on ACT: stays in
                # the ln/exp table set (no thrash against attention's Exp)
                lnv = small.tile([128, 8, 4], F32, tag="lnv", name=f"lnv{tp}")
                nc.scalar.activation(lnv[:], var[:], AF.Ln)
                a_ = small.tile([128, 8, 4], BF16, tag="a", name=f"a{tp}")
                nc.scalar.activation(a_[:], lnv[:], AF.Exp, scale=-0.5)
                nma = small.tile([128, 8, 4], BF16, tag="nma", name=f"nma{tp}")
                nc.vector.tensor_tensor(nma[:], mu[:], a_[:], OP.mult)
                yield
                # qn = q*a - mu*a
                nc.vector.tensor_tensor(
                    qk_nd[:], qk_nd[:],
                    a_[:, :, :, None].to_broadcast((128, 8, 4, D)), OP.mult)
                nc.vector.tensor_tensor(
                    qk_nd[:], qk_nd[:],
                    nma[:, :, :, None].to_broadcast((128, 8, 4, D)),
                    OP.subtract)
                yield
                # RoPE: out = qn*cos + rot_half(qn)*sin
                cs_lo = 8 * (tp % 2)
                cos_t = cos_sb[:, cs_lo:cs_lo + 8, :]
                sin_t = sin_sb[:, cs_lo:cs_lo + 8, :]
                cs = cos_t[:, :, None, :].to_broadcast((128, 8, 4, D))
                sn0 = sin_t[:, :, None, 0:32].to_broadcast((128, 8, 4, 32))
                sn1 = sin_t[:, :, None, 32:64].to_broadcast((128, 8, 4, 32))
                tmp = work.tile([128, 8, 4, D], BF16, tag="tmp",
                                name=f"tmp{tp}")
                nc.vector.tensor_tensor(tmp[:], qk_nd[:], cs, OP.mult)
                qk_r = qkrp.tile([128, 8, 4, D], BF16, tag="qkr",
                                 name=f"qkr{tp}")
                nc.vector.tensor_tensor(qk_r[:, :, :, 0:32],
                                        qk_nd[:, :, :, 32:64], sn0, OP.mult)
                nc.vector.tensor_tensor(qk_r[:, :, :, 0:32],
                                        tmp[:, :, :, 0:32],
                                        qk_r[:, :, :, 0:32], OP.subtract)
                yield
                nc.vector.tensor_tensor(qk_r[:, :, :, 32:64],
                                        qk_nd[:, :, :, 0:32], sn1, OP.mult)
                nc.vector.tensor_tensor(qk_r[:, :, :, 32:64],
                                        tmp[:, :, :, 32:64],
                                        qk_r[:, :, :, 32:64], OP.add)
                state[tp] = qk_r
                yield

            def transpose_gen(tp):
                """PE transposes: q,k -> [ch, n].  q is split per head into
                qpad's zero-padded bands; k keeps both heads (the S matmul
                contracts K=128 against the zero padding).  For tp 0,1 the
                PSUM->SBUF copies go on ACT (idle pre-attention) so DVE's
                RoPE backlog doesn't gate the first S matmuls."""
                qk_r = state.pop(tp)
                for j in range(8):
                    g = 8 * tp + j
                    col = 128 * g
                    ptqk = ps.tile([128, 2, 128], BF16, tag="ps",
                                   name=f"ptqk{g}")
                    nc.tensor.transpose(ptqk[:, 0, :], qk_r[:, j, 0:2, :],
                                        identr[:])
                    nc.tensor.transpose(ptqk[:, 1, :], qk_r[:, j, 2:4, :],
                                        identr[:])
                    # the qpad band copies go on DVE for ALL tps: ACT
                    # activation-copies into partition sub-ranges were
                    # observed to corrupt the data (b0-only NaNs).
                    nc.vector.tensor_copy(qpad[0:64, 0, col:col + 128],
                                          ptqk[0:64, 0, :])
                    nc.vector.tensor_copy(qpad[64:128, 1, col:col + 128],
                                          ptqk[64:128, 0, :])
                    if tp <= 1:
                        nc.scalar.activation(kT[:, col:col + 128],
                                             ptqk[:, 1, :], AF.Copy)
                    else:
                        nc.vector.tensor_copy(kT[:, col:col + 128],
                                              ptqk[:, 1, :])
                    if j % 2 == 1:
                        yield

            def attn_batch(b):
                """Attention for both local heads of batch b.  Matmuls are
                emitted in same-tiling-mode runs: all four S matmuls of a
                group (row-tiled 64x128, heads interleaved so the h0/h1
                pairs execute concurrently in row groups 0-1/2-3), then
                both exps, then the previous group's four AV matmuls
                (128x128 mode, batched with the fillers that follow) --
                two PE mode switches per group instead of four."""
                col0 = N * b

                def s_block(qt, g):
                    qs = col0 + 512 * qt
                    psS = [psSp.tile([128, 2, 512], F32, tag="pss",
                                     name=f"pS{b}{h}{qt}_{g}")
                           for h in range(2)]
                    for j in range(2):
                        kc = 2 * g + j
                        for h in range(2):
                            nc.tensor.matmul(
                                psS[h][:, j, :],
                                kT[:, col0 + 128 * kc:col0 + 128 * (kc + 1)],
                                qpad[:, h, qs:qs + 512],
                                start=True, stop=True)
                    es = []
                    for h in range(2):
                        e = expp.tile([128, 2, 512], BF16, tag="es",
                                      name=f"es{b}{h}{qt}_{g}")
                        nc.scalar.activation(e[:], psS[h][:], AF.Exp,
                                             scale=0.125)
                        es.append(e)
                    return es

                def av_block(qt, g, es, pav):
                    for h in range(2):
                        for j in range(2):
                            nc.tensor.matmul(
                                pav[h][:],
                                vext[:, 16 * b + 2 * g + j, h, :],
                                es[h][:, j, :],
                                start=(g == 0 and j == 0),
                                stop=(g == 7 and j == 1))

                def norm_qt(qt, pav):
                    """softmax denominators for BOTH heads -> one 128-lane
                    broadcast + Ln + Exp, then two DVE multiplies.  The
                    broadcast reads the zero-padded den_z so it runs as a
                    K=128 matmul (no PE tiling-mode switch)."""
                    qs = col0 + 512 * qt
                    # den copies on ACT: they gate the ln/exp right after
                    # on the same queue, while DVE may be several microsecs
                    # behind on filler work (cross-engine stall otherwise)
                    for h in range(2):
                        nc.scalar.activation(den_z[0:1, h, :],
                                             pav[h][64:65, :], AF.Copy)
                    pbc = ps.tile([128, 512], F32, tag="ps",
                                  name=f"pbc{b}{qt}")
                    for h in range(2):
                        nc.tensor.matmul(pbc[:],
                                         bones_sb[:, 128 * h:128 * (h + 1)],
                                         den_z[:, h, :],
                                         start=(h == 0), stop=(h == 1))
                    lnd = normp.tile([128, 512], F32, tag="lnd",
                                     name=f"lnd{b}{qt}")
                    nc.scalar.activation(lnd[:], pbc[:], AF.Ln)
                    bcr = normp.tile([128, 512], F32, tag="bcr",
                                     name=f"bcr{b}{qt}")
                    nc.scalar.activation(bcr[:], lnd[:], AF.Exp, scale=-1.0)
                    for h in range(2):
                        hof = D * h
                        nc.vector.tensor_tensor(
                            attn_outT[hof:hof + D, qs:qs + 512],
                            pav[h][0:64, :], bcr[hof:hof + D, :], OP.mult)

                for qt in range(4):
                    pav = [psav.tile([128, 512], F32, tag="av",
                                     name=f"pav{b}{h}{qt}")
                           for h in range(2)]
                    pend = None
                    for g in range(8):
                        es = s_block(qt, g)
                        if pend is not None:
                            av_block(qt, g - 1, pend, pav)
                        pend = es
                        if g < 7:
                            yield
                    av_block(qt, 7, pend, pav)
                    norm_qt(qt, pav)
                    yield

            # ---- AllToAll plumbing ---------------------------------------
            # chunk X covers attn_outT cols [CH0[X], CH0[X]+8*CHR[X]): dest
            # core j receives rows [CH0[X] + CHR[X]*j, +CHR[X]) -> its
            # output block X (row offset COFF[X]).  Five small collectives
            # so each hides under the next attention chunk; the last one
            # carries only 64 rows/core so its exposed tail is minimal.
            CH0 = [0, 1024, 2048, 2560, 3072, 3584]
            CHR = [128, 128, 64, 64, 64, 64]
            COFF = [0, 128, 256, 320, 384, 448]
            NCH = len(CH0)
            ccin = [dram.tile([N_CORES, 128, CHR[X]], BF16, name=f"ccin{X}")
                    for X in range(NCH)]
            ccout = [dram.tile([N_CORES, 128, CHR[X]], BF16, name=f"ccout{X}")
                     for X in range(NCH)]

            def emit_a2a(X):
                # single trigger (SP DMA triggers cost ~600ns each, serial)
                r = CHR[X]
                nc.sync.dma_start(
                    ccin[X][:].rearrange("j p n -> p j n"),
                    attn_outT[:, CH0[X]:CH0[X] + 8 * r].rearrange(
                        "p (j n) -> p j n", j=N_CORES))
                nc.gpsimd.collective_compute(
                    "AllToAll", OP.bypass,
                    replica_groups=[list(range(N_CORES))],
                    ins=[ccin[X][:].opt()], outs=[ccout[X][:].opt()])

            gat_tiles = {}

            def gat_fetch(nt):
                """gather block nt's collective output into SBUF.  Own tag
                per block (bufs=1, no slot reuse) so the DMA trigger never
                blocks the Sync queue waiting on a deferred outproj read."""
                r = CHR[nt]
                gat = freqs.tile([128, 8, r], BF16, tag=f"gat{nt}",
                                 name=f"gat{nt}", bufs=1)
                nc.sync.dma_start(gat[:],
                                  ccout[nt][:].rearrange("j p n -> p j n"))
                gat_tiles[nt] = gat

            def outproj_gen(nt):
                """project this core's CHR[nt]-row output block nt
                (gat_fetch(nt) must have been emitted already)."""
                r = CHR[nt]
                gat = gat_tiles.pop(nt)
                ob = work.tile([128, C], F32, tag="ob", name=f"ob{nt}")
                for hf in range(2):
                    po = ps.tile([128, 512], F32, tag="ps",
                                 name=f"po{nt}_{hf}")
                    for cc in range(8):
                        nc.tensor.matmul(
                            po[0:r, :],
                            gat[:, cc, :],
                            wp_sb[:, cc, 512 * hf:512 * (hf + 1)],
                            start=(cc == 0), stop=(cc == 7))
                        if cc == 3:
                            yield
                    nc.vector.tensor_tensor(
                        ob[0:r, 512 * hf:512 * (hf + 1)], po[0:r, :],
                        bias_sb[0:r, 512 * hf:512 * (hf + 1)], OP.add)
                    yield
                nc.sync.dma_start(
                    out_d.ap()[COFF[nt]:COFF[nt] + r, :], ob[0:r, :])
                yield

            def run_all(gen):
                for _ in gen:
                    pass

            def mix_steps(gen, fillers, steps, fill_per_step):
                """advance gen by `steps` yields, taking up to
                fill_per_step filler yields after each."""
                for _ in range(steps):
                    try:
                        next(gen)
                    except StopIteration:
                        break
                    took = 0
                    while fillers and took < fill_per_step:
                        try:
                            next(fillers[0])
                            took += 1
                        except StopIteration:
                            fillers.pop(0)

            def adv(gen, steps):
                for _ in range(steps):
                    try:
                        next(gen)
                    except StopIteration:
                        break

            # ---- emission schedule ---------------------------------------
            # PE heater: ~96 tiny matmuls fill the initial DMA wait and trip
            # HAM to full clock before the first real QKV matmul.
            pheat = ps.tile([128, 64], F32, tag="ps", name="pheat")
            for _ in range(96):
                nc.tensor.matmul(pheat[:], heat[:], heat[:, 0:64],
                                 start=True, stop=True)
            xt0 = load_xt(0)
            # cos/sin after xt0 (needed from ~38us; contiguous + small)
            nc.sync.dma_start(cos_sb[:], cos_d.ap().rearrange(
                "p (c d) -> p c d", d=D))
            nc.sync.dma_start(sin_sb[:], sin_d.ap().rearrange(
                "p (c d) -> p c d", d=D))
            run_all(pre_gen(0, xt0))
            emit_late_consts()
            xt1 = load_xt(1)
            # xt2's triggers go out right after xt1's (xload bufs=3: no WAR
            # wait can block the Sync queue here) so its data lands before
            # the tp2 bridge blocks below need it.
            xt2 = load_xt(2)
            run_all(pre_gen(1, xt1))
            xt3 = load_xt(3)
            run_all(transpose_gen(0))
            # bridge the pre->attention PE hole (tp1 RoPE tail on DVE
            # leaves the PE idle >3.4us otherwise -> HAM re-throttles right
            # as attention starts): give the PE tp2's first QKV blocks,
            # AFTER tp0's transposes so they aren't head-of-line blocked.
            p2 = pre_gen(2, xt2)
            adv(p2, 4)
            run_all(transpose_gen(1))
            # bias only feeds the tail outproj; keep it out of the early
            # DMA stream.
            nc.sync.dma_start(bias_sb[:], bias_d.ap())
            nc.sync.dma_start(wp_sb[:],
                              wpT_d.ap().rearrange("(co p) k -> p co k", p=128))
            # batch-0 attention yields after every 2-chunk group (8 per qt,
            # 32 total); one filler step per yield keeps the PE's in-order
            # queue dense inside each ACT-paced qt (HAM stays un-throttled).
            fillers = [p2, pre_gen(3, xt3),
                       transpose_gen(2), transpose_gen(3)]
            g0 = attn_batch(0)
            mix_steps(g0, fillers, 16, 1)       # b0 qt0,qt1
            emit_a2a(0)
            mix_steps(g0, fillers, 16, 1)       # b0 qt2,qt3
            run_all(g0)                         # drain: emits qt3's norm
            for f in fillers:
                run_all(f)
            emit_a2a(1)
            # batch-1 attention: per-qt collectives.  All outproj matmul
            # work is deferred to the tail so it hides the final AllToAll's
            # ~20us latency; only the cheap gat DMA triggers are emitted as
            # soon as each chunk's collective result is needed-by-able, and
            # every gat trigger lands BEFORE the next collective emission
            # (a later emission waits on the shared collective-output
            # semaphore and would serialize on it).
            # each gat trigger is emitted right after its own collective's
            # emission window (a trigger emitted after LATER collectives
            # waits on their completions too -- and emitting it long after
            # its collective was observed to corrupt the gather, so keep
            # trigger emission adjacent to its collective).
            gat_fetch(0)
            emit_a2a(1)
            # outproj blocks 0-2 run as b1 fillers (PE density: micro-idle
            # groups re-throttle HAM to half clock otherwise); each block
            # starts a few steps after its gat fetch so the fills don't
            # stall.  Blocks 3-5 run at the tail, 3/4 padding the final
            # AllToAll.
            g1 = attn_batch(1)
            opj = {}
            # fills start one qt after each gat fetch: a2a0 can complete as
            # late as ~190us, so opj0 must not head-of-line block the PE
            # queue at b1's first qt.
            fills = {10: 0, 11: 0, 12: 0, 13: 0, 14: 0,
                     18: 1, 19: 1, 20: 1, 21: 1, 22: 1,
                     26: 2, 27: 2, 28: 2, 29: 2, 30: 2}
            for step in range(32):
                try:
                    next(g1)
                except StopIteration:
                    break
                if step == 1:
                    gat_fetch(1)
                elif step == 7:
                    emit_a2a(2)                 # b1 qt0 rows
                elif step == 9:
                    gat_fetch(2)
                elif step == 15:
                    emit_a2a(3)                 # b1 qt1 rows
                elif step == 17:
                    gat_fetch(3)
                elif step == 23:
                    emit_a2a(4)                 # b1 qt2 rows
                elif step == 25:
                    gat_fetch(4)
                X = fills.get(step)
                if X is not None:
                    if X not in opj:
                        opj[X] = outproj_gen(X)
                    adv(opj[X], 1)
            run_all(g1)
            for X in range(3):
                run_all(opj[X])
            if _DBG:
                nc.sync.dma_start(dbg_ao_d.ap(), attn_outT[:])
                nc.sync.dma_start(dbg_qp_d.ap(), qpad[:])
                nc.sync.dma_start(dbg_kt_d.ap(), kT[:])
                nc.sync.dma_start(dbg_vx_d.ap(), vext[:])
            emit_a2a(5)                         # b1 qt3 rows
            run_all(outproj_gen(3))             # pad the final AllToAll
            run_all(outproj_gen(4))
            gat_fetch(5)
            run_all(outproj_gen(5))             # exposed: only 64 rows
    _split_excess_waits(nc)
    return nc


_NC_CACHE = {}


def _get_nc():
    if "nc" not in _NC_CACHE:
        _NC_CACHE["nc"] = build()
    return _NC_CACHE["nc"]


def _prep_inputs(x, w_qkv, w_proj, b_proj, freqs_cos, freqs_sin):
    x = np.asarray(x, dtype=np.float32)
    w_qkv = np.asarray(w_qkv, dtype=np.float32)
    w_proj = np.asarray(w_proj, dtype=np.float32)
    b_proj = np.asarray(b_proj, dtype=np.float32)
    bf = ml_dtypes.bfloat16

    def _swz(t):
        # [N, D] -> [p, c*D] matching the SBUF tile layout (n = c*128 + p)
        t = np.asarray(t, dtype=np.float32).reshape(16, 128, D)
        return np.ascontiguousarray(
            t.transpose(1, 0, 2).reshape(128, 16 * D)).astype(bf)

    cos = _swz(freqs_cos)
    sin = _swz(freqs_sin)

    xT = np.ascontiguousarray(x.reshape(NTOT, C).T).astype(bf)
    wpT = np.ascontiguousarray(w_proj.T).astype(bf)
    biasb = np.ascontiguousarray(
        np.broadcast_to(b_proj, (128, C))).astype(np.float32)
    ident = np.eye(128, dtype=np.float32)
    bones = np.zeros((128, 256), dtype=np.float32)
    bones[0, 0:64] = 1.0        # head0 lhsT: ones in cols 0-63
    bones[0, 192:256] = 1.0     # head1 lhsT: ones in cols 64-127
    bones = bones.astype(bf)

    in_maps = []
    for i in range(N_CORES):
        r0 = CPC * i
        wqkv = np.concatenate([w_qkv[r0:r0 + CPC],
                               w_qkv[C + r0:C + r0 + CPC],
                               w_qkv[2 * C + r0:2 * C + r0 + CPC]], axis=0)
        wqkvT = np.ascontiguousarray(wqkv.T).astype(bf)
        in_maps.append({
            "xT": xT, "wqkvT": wqkvT, "wpT": wpT,
            "biasb": biasb, "cosd": cos, "sind": sin, "identd": ident,
            "bonesd": bones,
        })
    return in_maps


def kernel(x, w_qkv, w_proj, b_proj, freqs_cos, freqs_sin):
    in_maps = _prep_inputs(x, w_qkv, w_proj, b_proj, freqs_cos, freqs_sin)
    nc = _get_nc()
    res = run_bass_kernel_spmd(nc, in_maps, core_ids=list(range(N_CORES)))
    CH0 = [0, 1024, 2048, 2560, 3072, 3584]
    CHR = [128, 128, 64, 64, 64, 64]
    COFF = [0, 128, 256, 320, 384, 448]
    full = np.empty((NTOT, C), dtype=np.float32)
    for i in range(N_CORES):
        o = res.results[i]["out"]
        for X in range(len(CH0)):
            r0 = CH0[X] + CHR[X] * i
            full[r0:r0 + CHR[X]] = o[COFF[X]:COFF[X] + CHR[X]]
    return full.reshape(B, N, C).astype(np.float32)

